# revision 10
# baseline (speedup 1.0000x reference)
"""Bass/Trainium2 kernel for nn_Attn (Bahdanau 'general' attention scoring).

Reference math:
    energies = einsum('sd,hd,h->s', enc, W, hidden) + b.hidden
    out      = softmax(energies)[None, None, :]

Factorization:
    v = W^T @ hidden (200-dim), energies = enc @ v (+ const; softmax cancels
    the constant b.hidden term, so b is dropped).

Distribution (8 NeuronCores, one TRN2 chip) — sequence sharding with a
replicated W. Profile-driven rationale: the per-execution collective
machinery on this runtime is a fixed ~65-75us chain (entry barrier ~40us
after the last core's first doorbell + ~11us first-collective setup), so
ALL heavy local work is hidden under that window and the only data
collective is made as small as possible:
  - Every core loads the FULL W and computes v = W^T @ hidden locally
    (no v collective). Contraction over h=8192 runs as DVE/GpSimd
    elementwise mult+reduce to [128, 200] partials, collapsed across
    partitions by one ones-matmul on the PE (which also broadcasts).
  - Core i owns seq slice [4096*i, 4096*(i+1)): energies e = enc_i @ v
    on the DVE, laid [128, 32] (s_local = p*32 + f).
  - Local softmax prep (all pre-collective): per-partition max m_p,
    q = exp(e - m_p), z_p = exp(m_p - m_c) with m_c the core max via
    gpsimd.partition_all_reduce, qz = q*z_p, s_c = sum_p sum_f qz.
  - ONE tiny AllGather of (m_c, s_c) pairs (16B total payload) replaces
    the 128KB energy AllReduce of the d-sharded variant: global
    M = max_c m_c, S = sum_c s_c*exp(m_c-M), and out_i = qz * alpha with
    alpha = exp(m_c - M)/S. Each core writes only its 4096-slice; the
    host concatenates the 8 shards.
  - A dependency-free warm-up AllGather rings the collective doorbell
    ~10.5us into execution (right after the fixed engine preamble) so
    the barrier+setup runs concurrently with the DMAs/compute.
"""

import numpy as np

N_CORES = 8
SEQ = 32768
D = 200
H = 8192
P = 128
KCH = H // P            # 64 h-chunks
S_LOCAL = SEQ // N_CORES    # 4096 positions per core
FSH = S_LOCAL // P      # 32 free positions per partition
# v-compute d-chunks, balanced for measured rates (DVE ~98G elem/s does
# mult for its chunks + ALL reduces; gpsimd ~37G elem/s does mult only).
D_CHUNKS_DVE = [(0, 45), (45, 90)]
D_CHUNKS_GPS = [(90, 145), (145, 200)]
# DMA order interleaves DVE/gpsimd chunks so both engines start early.
W_DMA_ORDER = [D_CHUNKS_DVE[0], D_CHUNKS_GPS[0], D_CHUNKS_DVE[1],
               D_CHUNKS_GPS[1]]


def build_kernel():
    import concourse.bacc as bacc
    import concourse.bass_isa as bass_isa
    import concourse.mybir as mybir
    import concourse.tile as tile

    fp32 = mybir.dt.float32
    nc = bacc.Bacc(
        "TRN2",
        target_bir_lowering=False,
        debug=False,
        num_devices=N_CORES,
    )

    # Host-prepacked layouts (see shard_inputs):
    #   encP [128, 32*200]: [p, f, d] with s_local = p*32 + f
    #   wP   [128, 200*64]: [p, d, k] with h = k*128 + p  (k innermost)
    #   hidP [128, 64]:     [p, k]    with h = k*128 + p
    encP = nc.dram_tensor("encP", [P, FSH * D], fp32, kind="ExternalInput")
    wP = nc.dram_tensor("wP", [P, D * KCH], fp32, kind="ExternalInput")
    hidP = nc.dram_tensor("hidP", [P, KCH], fp32, kind="ExternalInput")
    out = nc.dram_tensor("out", [S_LOCAL], fp32, kind="ExternalOutput")
    # Sink for the warm-up collective (kept live so it isn't DCE'd).
    warm_out = nc.dram_tensor("warm_out", [2, 4], fp32,
                              kind="ExternalOutput")

    rg8 = [list(range(N_CORES))]

    with tile.TileContext(nc) as tc:
        with (
            tc.tile_pool(name="const", bufs=1) as constp,
            tc.tile_pool(name="sb", bufs=1) as sb,
            tc.tile_pool(name="ps", bufs=1, space="PSUM") as ps,
            tc.tile_pool(name="dram", bufs=1, space="DRAM") as dram,
        ):
            # ---- warm-up collective, FIRST and with NO data dependencies:
            # rings the runtime's collective doorbell immediately after the
            # fixed engine preamble so the ~40us entry barrier + ~11us
            # first-collective setup run while the DMAs/compute proceed.
            warm_b = nc.inline_tensor(np.zeros((1, 4), np.float32),
                                      name="warm_src")
            warm_g = dram.tile([2, 4], fp32)
            nc.gpsimd.collective_compute(
                "AllGather",
                mybir.AluOpType.bypass,
                replica_groups=[[2 * i, 2 * i + 1] for i in range(N_CORES // 2)],
                ins=[warm_b.ap().opt()],
                outs=[warm_g[:].opt()],
            )
            ones = constp.tile([P, P], fp32)
            nc.vector.memset(ones[:], 1.0)

            # ---- loads (hid + W first: they gate the v chain) ----
            h_sb = sb.tile([P, KCH], fp32)
            nc.sync.dma_start(h_sb[:], hidP.ap())
            w_sb = sb.tile([P, D * KCH], fp32)
            for d0, d1 in W_DMA_ORDER:
                sl = slice(d0 * KCH, d1 * KCH)
                nc.sync.dma_start(w_sb[:, sl], wP.ap()[:, sl])
            enc_sb = sb.tile([P, FSH * D], fp32)
            half = FSH * D // 2
            nc.sync.dma_start(enc_sb[:, 0:half], encP.ap()[:, 0:half])
            nc.sync.dma_start(enc_sb[:, half:], encP.ap()[:, half:])

            # ---- v = W^T @ hidden: per-partition partials on DVE+GpSimd,
            # partition collapse + broadcast via one ones-matmul on PE ----
            w3 = w_sb[:].rearrange("p (d k) -> p d k", d=D)
            vpart = sb.tile([P, D], fp32)
            # gpsimd handles the multiplies for its d-chunks (it cannot do
            # free-axis reductions); DVE does its own mults + ALL reduces.
            prods = {}
            def v_mult(eng, d0, d1):
                dn = d1 - d0
                prod = sb.tile([P, dn * KCH], fp32, tag=f"prod{d0}")
                h_b = (
                    h_sb[:]
                    .rearrange("p k -> p () k")
                    .broadcast_to([P, dn, KCH])
                )
                eng.tensor_tensor(
                    out=prod[:].rearrange("p (d k) -> p d k", d=dn),
                    in0=w3[:, d0:d1, :],
                    in1=h_b,
                    op=mybir.AluOpType.mult,
                )
                prods[d0] = prod

            def v_reduce(d0, d1):
                dn = d1 - d0
                nc.vector.reduce_sum(
                    vpart[:, d0:d1],
                    prods[d0][:].rearrange("p (d k) -> p d k", d=dn),
                    axis=mybir.AxisListType.X,
                )

            # gpsimd mults first in emission (independent queue)
            for d0, d1 in D_CHUNKS_GPS:
                v_mult(nc.gpsimd, d0, d1)
            v_mult(nc.vector, *D_CHUNKS_DVE[0])
            v_reduce(*D_CHUNKS_DVE[0])
            v_mult(nc.vector, *D_CHUNKS_DVE[1])
            v_reduce(*D_CHUNKS_DVE[1])
            v_reduce(*D_CHUNKS_GPS[0])
            v_reduce(*D_CHUNKS_GPS[1])
            v_ps = ps.tile([P, D], fp32, tag="vps")
            nc.tensor.matmul(
                v_ps[:], lhsT=ones[:], rhs=vpart[:], start=True, stop=True
            )
            v_sb = sb.tile([P, D], fp32)
            nc.scalar.copy(v_sb[:], v_ps[:])

            # ---- energies e[p, f] = sum_d enc[p, f, d] * v[d] (DVE) ----
            enc3 = enc_sb[:].rearrange("p (f d) -> p f d", d=D)
            e_sb = sb.tile([P, FSH], fp32)
            for f0, f1 in ((0, FSH // 2), (FSH // 2, FSH)):
                fn = f1 - f0
                eprod = sb.tile([P, fn * D], fp32, tag="eprod", bufs=2)
                v_b = (
                    v_sb[:]
                    .rearrange("p d -> p () d")
                    .broadcast_to([P, fn, D])
                )
                nc.vector.tensor_tensor(
                    out=eprod[:].rearrange("p (f d) -> p f d", d=D),
                    in0=enc3[:, f0:f1, :],
                    in1=v_b,
                    op=mybir.AluOpType.mult,
                )
                nc.vector.reduce_sum(
                    e_sb[:, f0:f1],
                    eprod[:].rearrange("p (f d) -> p f d", d=D),
                    axis=mybir.AxisListType.X,
                )

            # ---- local softmax prep (all before the stats collective) ----
            negm_p = sb.tile([P, 1], fp32)
            nc.vector.reduce_max(negm_p[:], e_sb[:], axis=mybir.AxisListType.X,
                                 negate=True)
            m_p = sb.tile([P, 1], fp32)
            nc.vector.tensor_scalar_mul(m_p[:], negm_p[:], -1.0)
            q = sb.tile([P, FSH], fp32)
            s_p = sb.tile([P, 1], fp32)
            nc.scalar.activation(
                q[:], e_sb[:], mybir.ActivationFunctionType.Exp,
                bias=negm_p[:], scale=1.0, accum_out=s_p[:],
            )
            # core max m_c (broadcast to all partitions) via gpsimd
            m_c_bc = sb.tile([P, 1], fp32)
            nc.gpsimd.partition_all_reduce(
                m_c_bc[:], m_p[:], channels=P, reduce_op=bass_isa.ReduceOp.max
            )
            neg_mc = sb.tile([P, 1], fp32)
            nc.vector.tensor_scalar_mul(neg_mc[:], m_c_bc[:], -1.0)
            z_p = sb.tile([P, 1], fp32)
            nc.scalar.activation(
                z_p[:], m_p[:], mybir.ActivationFunctionType.Exp,
                bias=neg_mc[:], scale=1.0,
            )
            qz = sb.tile([P, FSH], fp32)
            nc.vector.tensor_scalar_mul(qz[:], q[:], z_p[:])
            sz_p = sb.tile([P, 1], fp32)
            nc.vector.tensor_tensor(sz_p[:], s_p[:], z_p[:],
                                    op=mybir.AluOpType.mult)
            s_c_bc = sb.tile([P, 1], fp32)
            nc.gpsimd.partition_all_reduce(
                s_c_bc[:], sz_p[:], channels=P, reduce_op=bass_isa.ReduceOp.add
            )
            stat_sb = sb.tile([1, 2], fp32)
            nc.scalar.copy(stat_sb[:, 0:1], m_c_bc[0:1, :])
            nc.scalar.copy(stat_sb[:, 1:2], s_c_bc[0:1, :])

            # ---- tiny stats AllGather: (m_c, s_c) pairs, 16B payload ----
            bounce = dram.tile([1, 2], fp32)
            statsg = dram.tile([1, 2 * N_CORES], fp32, addr_space="Shared")
            nc.sync.dma_start(bounce[:], stat_sb[:])
            nc.gpsimd.collective_compute(
                "AllGather",
                mybir.AluOpType.bypass,
                replica_groups=rg8,
                ins=[bounce[:].opt()],
                outs=[statsg[:].opt()],
            )
            sg = sb.tile([1, 2 * N_CORES], fp32)
            nc.sync.dma_start(sg[:], statsg[:])

            # ---- global combine: M = max_c m_c, S = sum_c s_c*exp(m_c-M),
            # alpha = exp(m_c - M) / S, out = qz * alpha ----
            sg2 = sg[:].rearrange("a (r two) -> a r two", two=2)
            m_view = sg2[:, :, 0]
            s_view = sg2[:, :, 1]
            negM = sb.tile([1, 1], fp32)
            nc.vector.reduce_max(negM[:], m_view, axis=mybir.AxisListType.X,
                                 negate=True)
            wexp = sb.tile([1, N_CORES], fp32)
            nc.scalar.activation(
                wexp[:], m_view, mybir.ActivationFunctionType.Exp,
                bias=negM[:], scale=1.0,
            )
            alpha_e = sb.tile([1, 1], fp32)
            nc.scalar.activation(
                alpha_e[:], m_c_bc[0:1, :], mybir.ActivationFunctionType.Exp,
                bias=negM[:], scale=1.0,
            )
            sw = sb.tile([1, N_CORES], fp32)
            nc.vector.tensor_tensor(sw[:], wexp[:], s_view,
                                    op=mybir.AluOpType.mult)
            S_sum = sb.tile([1, 1], fp32)
            nc.vector.reduce_sum(S_sum[:], sw[:], axis=mybir.AxisListType.X)
            rS = sb.tile([1, 1], fp32)
            nc.vector.reciprocal(rS[:], S_sum[:])
            alpha = sb.tile([1, 1], fp32)
            nc.vector.tensor_tensor(alpha[:], alpha_e[:], rS[:],
                                    op=mybir.AluOpType.mult)
            alpha_bc = sb.tile([P, 1], fp32)
            nc.gpsimd.partition_broadcast(alpha_bc[:], alpha[:])
            o_sb = sb.tile([P, FSH], fp32)
            nc.vector.tensor_scalar_mul(o_sb[:], qz[:], alpha_bc[:])
            nc.sync.dma_start(out.ap().rearrange("(p f) -> p f", p=P), o_sb[:])

            # Keep the warm-up collective live. Emitted LAST so this DMA
            # (which waits on the warm AllGather) never blocks earlier
            # work queued behind it on the same engine.
            nc.scalar.dma_start(warm_out.ap(), warm_g[:])

    nc.compile()
    return nc


def shard_inputs(hidden, encoder_outputs, W, b):
    hidden = np.asarray(hidden, dtype=np.float32)
    enc = np.asarray(encoder_outputs, dtype=np.float32)
    W = np.asarray(W, dtype=np.float32)
    # wP: [p, d, k] with h = k*128 + p
    wP = np.ascontiguousarray(
        W.reshape(KCH, P, D).transpose(1, 2, 0)
    ).reshape(P, D * KCH)
    hidP = np.ascontiguousarray(hidden.reshape(KCH, P).T)  # [p, k]
    in_maps = []
    for i in range(N_CORES):
        shard = enc[i * S_LOCAL:(i + 1) * S_LOCAL]          # [4096, 200]
        encP_i = np.ascontiguousarray(shard).reshape(P, FSH * D)
        in_maps.append({"encP": encP_i, "wP": wP, "hidP": hidP})
    return in_maps


_NC_CACHE = {}


def _get_nc():
    if "nc" not in _NC_CACHE:
        _NC_CACHE["nc"] = build_kernel()
    return _NC_CACHE["nc"]


def kernel(hidden, encoder_outputs, W, b):
    from concourse import bass_utils

    nc = _get_nc()
    in_maps = shard_inputs(hidden, encoder_outputs, W, b)
    res = bass_utils.run_bass_kernel_spmd(
        nc, in_maps, core_ids=list(range(N_CORES))
    )
    out = np.concatenate(
        [np.asarray(res.results[c]["out"], dtype=np.float32)
         for c in range(N_CORES)]
    )
    return out.reshape(1, 1, SEQ)


# revision 11
# speedup vs baseline: 1.0845x; 1.0845x over previous
"""Bass/Trainium2 kernel for nn_Attn (Bahdanau 'general' attention scoring).

Reference math:
    energies = einsum('sd,hd,h->s', enc, W, hidden) + b.hidden
    out      = softmax(energies)[None, None, :]

Factorization:
    v = W^T @ hidden (200-dim), energies = enc @ v (+ const; softmax cancels
    the constant b.hidden term, so b is dropped).

Distribution (8 NeuronCores, one TRN2 chip) — sequence sharding with a
replicated W. Profile-driven rationale: the per-execution collective
machinery on this runtime is a fixed ~65-78us chain (entry barrier ~40us
after the last core's first collective doorbell + ~11us first-collective
setup + warm-op), so ALL heavy local work is hidden under that window
and the only data collective is made as small as possible:
  - Every core loads the FULL W and computes v = W^T @ hidden locally
    (no v collective): elementwise mult+reduce on the DVE over
    [128, d, 64] chunks (small-first chunk sizes so the DVE starts as
    soon as the first W chunk lands), partition-collapsed+broadcast by
    one ones-matmul on the PE. GpSimd is NOT used for tensor work: DVE
    and GpSimd share SBUF ports, so concurrent elementwise work caps at
    the same ~116G elem/s as DVE alone.
  - Core i owns seq slice [4096*i, 4096*(i+1)): energies e = enc_i @ v
    on the DVE, laid [128, 32] (s_local = p*32 + f).
  - Local softmax, all pre-collective: m_c = core max via DVE reduce +
    gpsimd.partition_all_reduce(max); q = exp(e - m_c) with per-row
    accumulation, summed across partitions by partition_all_reduce(add).
  - ONE tiny AllGather of (m_c, s_c) pairs (16B total payload) replaces
    a 128KB energy AllReduce: global M = max_c m_c,
    S = sum_c s_c*exp(m_c-M), out_i = q * alpha with
    alpha = exp(m_c - M)/S. Each core writes only its 4096-slice; the
    host concatenates the 8 shards.
  - A dependency-free warm-up AllGather rings the collective doorbell
    right after the fixed engine preamble so the barrier+setup runs
    concurrently with the DMAs/compute. Its sink DMA is pinned to the
    end of the schedule via tile_wait_until so the Tile scheduler
    cannot place it ahead of real work on the same queue (its wait on
    the warm collective would stall that queue for ~50us).
"""

import numpy as np

N_CORES = 8
SEQ = 32768
D = 200
H = 8192
P = 128
KCH = H // P            # 64 h-chunks
S_LOCAL = SEQ // N_CORES    # 4096 positions per core
FSH = S_LOCAL // P      # 32 free positions per partition
# v-compute d-chunks, small-first so the DVE starts on the first W DMA.
W_CHUNKS = [(0, 16), (16, 48), (48, 88), (88, 144), (144, 200)]


def build_kernel():
    import concourse.bacc as bacc
    import concourse.bass_isa as bass_isa
    import concourse.mybir as mybir
    import concourse.tile as tile

    fp32 = mybir.dt.float32
    nc = bacc.Bacc(
        "TRN2",
        target_bir_lowering=False,
        debug=False,
        num_devices=N_CORES,
    )

    # Host-prepacked layouts (see shard_inputs):
    #   encP [128, 32*200]: [p, f, d] with s_local = p*32 + f
    #   wP   [128, 200*64]: [p, d, k] with h = k*128 + p  (k innermost)
    #   hidP [128, 64]:     [p, k]    with h = k*128 + p
    encP = nc.dram_tensor("encP", [P, FSH * D], fp32, kind="ExternalInput")
    wP = nc.dram_tensor("wP", [P, D * KCH], fp32, kind="ExternalInput")
    hidP = nc.dram_tensor("hidP", [P, KCH], fp32, kind="ExternalInput")
    out = nc.dram_tensor("out", [S_LOCAL], fp32, kind="ExternalOutput")
    # Sink for the warm-up collective (kept live so it isn't DCE'd).
    warm_out = nc.dram_tensor("warm_out", [2, 4], fp32,
                              kind="ExternalOutput")

    rg8 = [list(range(N_CORES))]

    with tile.TileContext(nc) as tc:
        with (
            tc.tile_pool(name="const", bufs=1) as constp,
            tc.tile_pool(name="sb", bufs=1) as sb,
            tc.tile_pool(name="ps", bufs=1, space="PSUM") as ps,
            tc.tile_pool(name="dram", bufs=1, space="DRAM") as dram,
        ):
            # ---- warm-up collective, FIRST and with NO data dependencies:
            # rings the runtime's collective doorbell immediately after the
            # fixed engine preamble so the ~40us entry barrier + ~11us
            # first-collective setup run while the DMAs/compute proceed.
            warm_b = nc.inline_tensor(np.zeros((1, 4), np.float32),
                                      name="warm_src")
            warm_g = dram.tile([2, 4], fp32)
            nc.gpsimd.collective_compute(
                "AllGather",
                mybir.AluOpType.bypass,
                replica_groups=[[2 * i, 2 * i + 1] for i in range(N_CORES // 2)],
                ins=[warm_b.ap().opt()],
                outs=[warm_g[:].opt()],
            )

            ones = constp.tile([P, P], fp32)
            nc.vector.memset(ones[:], 1.0)

            # ---- loads (hid + W first: they gate the v chain) ----
            h_sb = sb.tile([P, KCH], fp32)
            nc.sync.dma_start(h_sb[:], hidP.ap())
            w_sb = sb.tile([P, D * KCH], fp32)
            for d0, d1 in W_CHUNKS:
                sl = slice(d0 * KCH, d1 * KCH)
                nc.sync.dma_start(w_sb[:, sl], wP.ap()[:, sl])
            enc_sb = sb.tile([P, FSH * D], fp32)
            half = FSH * D // 2
            nc.sync.dma_start(enc_sb[:, 0:half], encP.ap()[:, 0:half])
            nc.sync.dma_start(enc_sb[:, half:], encP.ap()[:, half:])

            # ---- v = W^T @ hidden: per-partition partials on the DVE,
            # partition collapse + broadcast via one ones-matmul on PE ----
            w3 = w_sb[:].rearrange("p (d k) -> p d k", d=D)
            vpart = sb.tile([P, D], fp32)
            for d0, d1 in W_CHUNKS:
                dn = d1 - d0
                prod = sb.tile([P, dn * KCH], fp32, tag="prod", bufs=2)
                h_b = (
                    h_sb[:]
                    .rearrange("p k -> p () k")
                    .broadcast_to([P, dn, KCH])
                )
                nc.vector.tensor_tensor(
                    out=prod[:].rearrange("p (d k) -> p d k", d=dn),
                    in0=w3[:, d0:d1, :],
                    in1=h_b,
                    op=mybir.AluOpType.mult,
                )
                nc.vector.reduce_sum(
                    vpart[:, d0:d1],
                    prod[:].rearrange("p (d k) -> p d k", d=dn),
                    axis=mybir.AxisListType.X,
                )
            v_ps = ps.tile([P, D], fp32, tag="vps")
            nc.tensor.matmul(
                v_ps[:], lhsT=ones[:], rhs=vpart[:], start=True, stop=True
            )
            v_sb = sb.tile([P, D], fp32)
            nc.scalar.copy(v_sb[:], v_ps[:])

            # ---- energies e[p, f] = sum_d enc[p, f, d] * v[d] (DVE) ----
            enc3 = enc_sb[:].rearrange("p (f d) -> p f d", d=D)
            e_sb = sb.tile([P, FSH], fp32)
            for f0, f1 in ((0, FSH // 2), (FSH // 2, FSH)):
                fn = f1 - f0
                eprod = sb.tile([P, fn * D], fp32, tag="eprod", bufs=2)
                v_b = (
                    v_sb[:]
                    .rearrange("p d -> p () d")
                    .broadcast_to([P, fn, D])
                )
                nc.vector.tensor_tensor(
                    out=eprod[:].rearrange("p (f d) -> p f d", d=D),
                    in0=enc3[:, f0:f1, :],
                    in1=v_b,
                    op=mybir.AluOpType.mult,
                )
                nc.vector.reduce_sum(
                    e_sb[:, f0:f1],
                    eprod[:].rearrange("p (f d) -> p f d", d=D),
                    axis=mybir.AxisListType.X,
                )

            # ---- local softmax prep (all before the stats collective):
            # m_c = max(e) over the whole core, q = exp(e - m_c),
            # s_c = sum(q). PAR writes land in adjacent columns so the
            # bounce DMA reads (m_c, s_c) directly with no copies. ----
            m_p = sb.tile([P, 1], fp32)
            nc.vector.reduce_max(m_p[:], e_sb[:], axis=mybir.AxisListType.X)
            stat2 = sb.tile([P, 2], fp32)
            nc.gpsimd.partition_all_reduce(
                stat2[:, 0:1], m_p[:], channels=P,
                reduce_op=bass_isa.ReduceOp.max,
            )
            neg_mc = sb.tile([P, 1], fp32)
            nc.vector.tensor_scalar_mul(neg_mc[:], stat2[:, 0:1], -1.0)
            q = sb.tile([P, FSH], fp32)
            s_p = sb.tile([P, 1], fp32)
            nc.scalar.activation(
                q[:], e_sb[:], mybir.ActivationFunctionType.Exp,
                bias=neg_mc[:], scale=1.0, accum_out=s_p[:],
            )
            nc.gpsimd.partition_all_reduce(
                stat2[:, 1:2], s_p[:], channels=P,
                reduce_op=bass_isa.ReduceOp.add,
            )

            # ---- tiny stats AllGather: (m_c, s_c) pairs, 16B payload ----
            bounce = dram.tile([1, 2], fp32)
            statsg = dram.tile([1, 2 * N_CORES], fp32, addr_space="Shared")
            nc.sync.dma_start(bounce[:], stat2[0:1, :])
            nc.gpsimd.collective_compute(
                "AllGather",
                mybir.AluOpType.bypass,
                replica_groups=rg8,
                ins=[bounce[:].opt()],
                outs=[statsg[:].opt()],
            )
            sg = sb.tile([1, 2 * N_CORES], fp32)
            nc.sync.dma_start(sg[:], statsg[:])

            # ---- global combine: M = max_c m_c, S = sum_c s_c*exp(m_c-M),
            # alpha = exp(m_c - M) / S, out = q * alpha ----
            sg2 = sg[:].rearrange("a (r two) -> a r two", two=2)
            m_view = sg2[:, :, 0]
            s_view = sg2[:, :, 1]
            negM = sb.tile([1, 1], fp32)
            nc.vector.reduce_max(negM[:], m_view, axis=mybir.AxisListType.X,
                                 negate=True)
            wexp = sb.tile([1, N_CORES], fp32)
            nc.scalar.activation(
                wexp[:], m_view, mybir.ActivationFunctionType.Exp,
                bias=negM[:], scale=1.0,
            )
            alpha_e = sb.tile([1, 1], fp32)
            nc.scalar.activation(
                alpha_e[:], stat2[0:1, 0:1], mybir.ActivationFunctionType.Exp,
                bias=negM[:], scale=1.0,
            )
            sw = sb.tile([1, N_CORES], fp32)
            nc.vector.tensor_tensor(sw[:], wexp[:], s_view,
                                    op=mybir.AluOpType.mult)
            S_sum = sb.tile([1, 1], fp32)
            nc.vector.reduce_sum(S_sum[:], sw[:], axis=mybir.AxisListType.X)
            rS = sb.tile([1, 1], fp32)
            nc.vector.reciprocal(rS[:], S_sum[:])
            alpha = sb.tile([1, 1], fp32)
            nc.vector.tensor_tensor(alpha[:], alpha_e[:], rS[:],
                                    op=mybir.AluOpType.mult)
            alpha_bc = sb.tile([P, 1], fp32)
            nc.gpsimd.partition_broadcast(alpha_bc[:], alpha[:])
            o_sb = sb.tile([P, FSH], fp32)
            nc.vector.tensor_scalar_mul(o_sb[:], q[:], alpha_bc[:])
            nc.sync.dma_start(out.ap().rearrange("(p f) -> p f", p=P), o_sb[:])

            # Keep the warm-up collective live. tile_wait_until pins it to
            # the end of the Tile scheduler's timeline so its wait on the
            # warm AllGather never stalls real work queued after it.
            with tc.tile_wait_until(1.0):
                nc.scalar.dma_start(warm_out.ap(), warm_g[:])

    nc.compile()
    return nc


def shard_inputs(hidden, encoder_outputs, W, b):
    hidden = np.asarray(hidden, dtype=np.float32)
    enc = np.asarray(encoder_outputs, dtype=np.float32)
    W = np.asarray(W, dtype=np.float32)
    # wP: [p, d, k] with h = k*128 + p
    wP = np.ascontiguousarray(
        W.reshape(KCH, P, D).transpose(1, 2, 0)
    ).reshape(P, D * KCH)
    hidP = np.ascontiguousarray(hidden.reshape(KCH, P).T)  # [p, k]
    in_maps = []
    for i in range(N_CORES):
        shard = enc[i * S_LOCAL:(i + 1) * S_LOCAL]          # [4096, 200]
        encP_i = np.ascontiguousarray(shard).reshape(P, FSH * D)
        in_maps.append({"encP": encP_i, "wP": wP, "hidP": hidP})
    return in_maps


_NC_CACHE = {}


def _get_nc():
    if "nc" not in _NC_CACHE:
        _NC_CACHE["nc"] = build_kernel()
    return _NC_CACHE["nc"]


def kernel(hidden, encoder_outputs, W, b):
    from concourse import bass_utils

    nc = _get_nc()
    in_maps = shard_inputs(hidden, encoder_outputs, W, b)
    res = bass_utils.run_bass_kernel_spmd(
        nc, in_maps, core_ids=list(range(N_CORES))
    )
    out = np.concatenate(
        [np.asarray(res.results[c]["out"], dtype=np.float32)
         for c in range(N_CORES)]
    )
    return out.reshape(1, 1, SEQ)
